# Initial kernel scaffold
#
"""MultiHeadLatentAttention TRN2 kernel.

Sharding: head-parallel across 8 NeuronCores (2 heads/core x 2 batches = 4
independent (batch, head) units per core). Everything on-device runs in a
"transposed" layout (feature dim on SBUF partitions, sequence on the free
dim) so scores are produced key-major and softmax numerators come straight
out of the Scalar engine in the layout the PV matmul needs.

Host-side algebraic folds (all exact):
  - LN mean:   Wdkv centered column-wise  -> ckv_raw is zero-mean over L.
  - rotate_half: premultiplied weight copies (R @ W) -> RoPE is two fused
    multiply-adds against precomputed cos/sin tables.
  - Wq @ Wuk^T absorbed (as in the reference), Wo folded into Wuv.
"""

import os
import sys

for _p in ("/opt/trn_rl_repo", "/root/.axon_site/_ro/trn_rl_repo"):
    if os.path.isdir(_p) and _p not in sys.path:
        sys.path.insert(0, _p)

import numpy as np
import ml_dtypes

import concourse.bass as bass
import concourse.mybir as mybir
from concourse.tile import TileContext
from concourse.bass_utils import run_bass_kernel_spmd

B, S, D, H, HD, L = 2, 2048, 2048, 16, 128, 128
P = 128
NCORES = 8
HPC = H // NCORES      # heads per core
NU = HPC * B           # (head, batch) units per core
NJ = S // P            # 128-row blocks along the sequence

F32 = mybir.dt.float32
F32R = mybir.dt.float32r
BF16 = mybir.dt.bfloat16
AF = mybir.ActivationFunctionType
ALU = mybir.AluOpType

MASK_NEG = -480.0      # causal mask additive constant, pre 1/16 scale
SCALE = 1.0 / 16.0     # 1/sqrt(2*L)

_prog_cache = {}


def _split_waits(nc, limit=1):
    """walrus enforces a small per-instruction sync-wait budget; move excess
    waits onto preceding NoOps on the same engine queue."""
    n = 0
    for f in nc.m.functions:
        for blk in f.blocks:
            out = []
            for inst in blk.instructions:
                si = inst.sync_info
                waits = list(si.on_wait) if si and si.on_wait else []
                if len(waits) > limit:
                    for k, w in enumerate(waits[:-limit]):
                        nop = mybir.InstNoOp(name=f"{inst.name}-ws{k}")
                        nop.engine = inst.engine
                        nop.sync_info = mybir.SyncInfo(on_wait=[w], on_update=[])
                        out.append(nop)
                        n += 1
                    si.on_wait = waits[-limit:]
                out.append(inst)
            blk.instructions = out
    return n


def _chunks(lo, hi, step):
    return [(o, min(step, hi - o)) for o in range(lo, hi, step)]


def _build(with_qbias=False):
    nc = bass.Bass(target_bir_lowering=False)

    xt_d = nc.dram_tensor("xt", [NU, P, S], F32R, kind="ExternalInput")
    ws_d = nc.dram_tensor("ws", [HPC, 7, P, P], F32R, kind="ExternalInput")
    gb_d = nc.dram_tensor("gb", [HPC, 2, P, 1], F32, kind="ExternalInput")
    cos_d = nc.dram_tensor("cosT", [P, S], F32, kind="ExternalInput")
    sin_d = nc.dram_tensor("sinT", [P, S], F32, kind="ExternalInput")
    tri_d = nc.dram_tensor("tri", [P, P], BF16, kind="ExternalInput")
    idb_d = nc.dram_tensor("idb", [P, P], BF16, kind="ExternalInput")
    idr_d = nc.dram_tensor("idr", [P, P], F32R, kind="ExternalInput")
    one_d = nc.dram_tensor("ones", [P, 1], F32R, kind="ExternalInput")
    if with_qbias:
        qb_d = nc.dram_tensor("qb", [HPC, 2, P, 1], F32, kind="ExternalInput")

    oo_d = nc.dram_tensor("out_o", [NU, S, P], F32, kind="ExternalOutput")
    oc_d = nc.dram_tensor("out_ckv", [NU, S, P], F32, kind="ExternalOutput")
    ok_d = nc.dram_tensor("out_kr", [NU, S, P], F32, kind="ExternalOutput")

    with TileContext(nc) as tc:
        with tc.tile_pool(name="const", bufs=1) as cst, \
             tc.tile_pool(name="sb", bufs=2) as sb, \
             tc.tile_pool(name="sb1", bufs=1) as sb1, \
             tc.tile_pool(name="pt", bufs=1) as ptp, \
             tc.tile_pool(name="st", bufs=4) as st, \
             tc.tile_pool(name="psc", bufs=2, space="PSUM") as psc, \
             tc.tile_pool(name="pp", bufs=4, space="PSUM") as pp:

            # ---- constants ----
            cosT = cst.tile([P, S], F32)
            sinT = cst.tile([P, S], F32)
            tri = cst.tile([P, P], BF16)
            idb = cst.tile([P, P], BF16)
            idr = cst.tile([P, P], F32R)
            ones = cst.tile([P, 1], F32R)
            nc.sync.dma_start(out=cosT[:], in_=cos_d[:])
            nc.sync.dma_start(out=sinT[:], in_=sin_d[:])
            nc.sync.dma_start(out=tri[:], in_=tri_d[:])
            nc.sync.dma_start(out=idb[:], in_=idb_d[:])
            nc.sync.dma_start(out=idr[:], in_=idr_d[:])
            nc.sync.dma_start(out=ones[:], in_=one_d[:])

            W = {}
            gam, bet = {}, {}
            qb1, qb2 = {}, {}
            wnames = ["dkv", "kr", "krr", "abs", "qr", "qrr", "uv2"]
            for hl in range(HPC):
                for wi, wn in enumerate(wnames):
                    t = cst.tile([P, P], F32R, tag=f"w{hl}{wn}")
                    nc.sync.dma_start(out=t[:], in_=ws_d[hl, wi])
                    W[hl, wn] = t
                gam[hl] = cst.tile([P, 1], F32, tag=f"g{hl}")
                bet[hl] = cst.tile([P, 1], F32, tag=f"b{hl}")
                nc.sync.dma_start(out=gam[hl][:], in_=gb_d[hl, 0])
                nc.sync.dma_start(out=bet[hl][:], in_=gb_d[hl, 1])
                if with_qbias:
                    qb1[hl] = cst.tile([P, 1], F32, tag=f"q1{hl}")
                    qb2[hl] = cst.tile([P, 1], F32, tag=f"q2{hl}")
                    nc.sync.dma_start(out=qb1[hl][:], in_=qb_d[hl, 0])
                    nc.sync.dma_start(out=qb2[hl][:], in_=qb_d[hl, 1])

            for u in range(NU):
                hl, b = u // B, u % B

                # ---------- phase 1: load + projections ----------
                xts = sb.tile([P, S], F32R, tag="xts")
                nc.sync.dma_start(out=xts[:], in_=xt_d[u])

                ckvT = sb.tile([P, S], F32R, tag="ckvT")
                krT = sb.tile([P, S], F32R, tag="krT")
                q1T = sb.tile([P, S], F32R, tag="q1T")
                qrT = sb.tile([P, S], F32R, tag="qrT")

                # ckv chain with LayerNorm (mean pre-folded into weights)
                for off, w in _chunks(0, S, 512):
                    xc = pp.tile([P, 512], F32, tag="pp")[:, :w]
                    nc.tensor.matmul(xc, W[hl, "dkv"][:], xts[:, off:off + w],
                                     start=True, stop=True)
                    sq = st.tile([P, 512], F32R, tag="sq")[:, :w]
                    nc.scalar.activation(sq, xc, AF.Square)
                    var = pp.tile([P, 512], F32, tag="pp")[0:1, :w]
                    nc.tensor.matmul(var, ones[:], sq, start=True, stop=True)
                    lnv = st.tile([1, 512], F32, tag="lnv")[:, :w]
                    nc.scalar.activation(lnv, var, AF.Ln, scale=1.0 / L, bias=1e-5)
                    rst = st.tile([1, 512], F32, tag="rst")[:, :w]
                    nc.scalar.activation(rst, lnv, AF.Exp, scale=-0.5)
                    rbc = st.tile([P, 512], F32, tag="rbc")[:, :w]
                    nc.gpsimd.partition_broadcast(rbc, rst)
                    t1 = st.tile([P, 512], F32, tag="t1")[:, :w]
                    nc.vector.tensor_tensor(out=t1, in0=xc, in1=rbc, op=ALU.mult)
                    nc.vector.tensor_scalar(out=ckvT[:, off:off + w], in0=t1,
                                            scalar1=gam[hl][:], scalar2=bet[hl][:],
                                            op0=ALU.mult, op1=ALU.add)

                # decoupled RoPE key
                for off, w in _chunks(0, S, 512):
                    ka = pp.tile([P, 512], F32, tag="pp")[:, :w]
                    nc.tensor.matmul(ka, W[hl, "kr"][:], xts[:, off:off + w],
                                     start=True, stop=True)
                    kb = pp.tile([P, 512], F32, tag="pp")[:, :w]
                    nc.tensor.matmul(kb, W[hl, "krr"][:], xts[:, off:off + w],
                                     start=True, stop=True)
                    k1 = st.tile([P, 512], F32, tag="k1")[:, :w]
                    nc.vector.tensor_tensor(out=k1, in0=ka, in1=cosT[:, off:off + w],
                                            op=ALU.mult)
                    k2 = st.tile([P, 512], F32, tag="k2")[:, :w]
                    nc.vector.tensor_tensor(out=k2, in0=kb, in1=sinT[:, off:off + w],
                                            op=ALU.mult)
                    nc.gpsimd.tensor_tensor(out=krT[:, off:off + w], in0=k1, in1=k2,
                                            op=ALU.add)

                # absorbed no-rope query
                for off, w in _chunks(0, S, 512):
                    qp = pp.tile([P, 512], F32, tag="pp")[:, :w]
                    nc.tensor.matmul(qp, W[hl, "abs"][:], xts[:, off:off + w],
                                     start=True, stop=True)
                    nc.vector.tensor_copy(q1T[:, off:off + w], qp)

                # RoPE query from latent
                for off, w in _chunks(0, S, 512):
                    qa = pp.tile([P, 512], F32, tag="pp")[:, :w]
                    nc.tensor.matmul(qa, W[hl, "qr"][:], ckvT[:, off:off + w],
                                     start=True, stop=True)
                    qb = pp.tile([P, 512], F32, tag="pp")[:, :w]
                    nc.tensor.matmul(qb, W[hl, "qrr"][:], ckvT[:, off:off + w],
                                     start=True, stop=True)
                    if with_qbias:
                        qa2 = st.tile([P, 512], F32, tag="qa2")[:, :w]
                        nc.vector.tensor_scalar(out=qa2, in0=qa, scalar1=qb1[hl][:],
                                                scalar2=None, op0=ALU.add)
                        qb2s = st.tile([P, 512], F32, tag="qb2")[:, :w]
                        nc.vector.tensor_scalar(out=qb2s, in0=qb, scalar1=qb2[hl][:],
                                                scalar2=None, op0=ALU.add)
                        qa, qb = qa2, qb2s
                    q1 = st.tile([P, 512], F32, tag="qq1")[:, :w]
                    nc.vector.tensor_tensor(out=q1, in0=qa, in1=cosT[:, off:off + w],
                                            op=ALU.mult)
                    q2 = st.tile([P, 512], F32, tag="qq2")[:, :w]
                    nc.vector.tensor_tensor(out=q2, in0=qb, in1=sinT[:, off:off + w],
                                            op=ALU.mult)
                    nc.gpsimd.tensor_tensor(out=qrT[:, off:off + w], in0=q1, in1=q2,
                                            op=ALU.add)

                # values (Wo folded in) + ones column for the softmax denominator
                v2e = sb.tile([P, NJ * 129], BF16, tag="v2e")
                nc.vector.memset(v2e[:], 1.0)
                for j in range(NJ):
                    vp = pp.tile([P, 512], F32, tag="pp")[:, :P]
                    nc.tensor.matmul(vp, ckvT[:, j * P:(j + 1) * P],
                                     W[hl, "uv2"][:], start=True, stop=True)
                    nc.vector.tensor_copy(v2e[:, j * 129:j * 129 + P], vp)

                # natural-layout ckv / k_r outputs via PE transpose
                ckvn = sb1.tile([P, S], F32, tag="ckvn")
                krn = sb1.tile([P, S], F32, tag="krn")
                for src, dst in ((ckvT, ckvn), (krT, krn)):
                    for off, w in _chunks(0, S, 512):
                        tp = pp.tile([P, 512], F32, tag="pp")[:, :w]
                        for k in range(0, w, P):
                            nc.tensor.transpose(tp[:, k:k + P],
                                                src[:, off + k:off + k + P], idr[:])
                        nc.vector.tensor_copy(dst[:, off:off + w], tp)

                # ---------- phase 2: attention ----------
                pts = []
                for j in range(NJ):
                    wsc = S - j * P
                    ptj = ptp.tile([P, wsc], BF16, tag=f"pt{j}")
                    pts.append(ptj)
                    for soff, sw in _chunks(j * P, S, 1024):
                        sc = psc.tile([P, 1024], F32, tag="sc")[:, :sw]
                        has_diag = soff == j * P
                        for o2, w2 in _chunks(soff, soff + sw, 512):
                            rel = o2 - soff
                            last = o2 + w2 == soff + sw
                            nc.tensor.matmul(sc[:, rel:rel + w2],
                                             ckvT[:, j * P:(j + 1) * P],
                                             q1T[:, o2:o2 + w2],
                                             start=True, stop=False)
                            nc.tensor.matmul(sc[:, rel:rel + w2],
                                             krT[:, j * P:(j + 1) * P],
                                             qrT[:, o2:o2 + w2],
                                             start=False,
                                             stop=(not has_diag or rel > 0) and last)
                        if has_diag:
                            nc.tensor.matmul(sc[:, 0:P], tri[:], idb[:],
                                             start=False, stop=True,
                                             skip_group_check=True)
                        nc.scalar.activation(ptj[:, soff - j * P:soff - j * P + sw],
                                             sc, AF.Exp, scale=SCALE)

                    # attention output + Z for query block i = j
                    op = pp.tile([P, 512], F32, tag="pp")[:, :129]
                    for jj in range(j + 1):
                        nc.tensor.matmul(op, pts[jj][:, (j - jj) * P:(j - jj + 1) * P],
                                         v2e[:, jj * 129:(jj + 1) * 129],
                                         start=(jj == 0), stop=(jj == j))
                    rz = st.tile([P, 1], F32, tag="rz")
                    nc.vector.reciprocal(rz[:], op[:, 128:129])
                    if j == 0:
                        ostg = sb1.tile([P, S], F32, tag="ostg")
                    nc.vector.tensor_scalar(out=ostg[:, j * P:(j + 1) * P],
                                            in0=op[:, 0:P], scalar1=rz[:],
                                            scalar2=None, op0=ALU.mult)

                # ---------- phase 3: stores ----------
                for stg, dd in ((ostg, oo_d), (ckvn, oc_d), (krn, ok_d)):
                    nc.sync.dma_start(
                        out=dd[u].rearrange("(j p) e -> p j e", p=P),
                        in_=stg[:].rearrange("p (j e) -> p j e", j=NJ))

    _split_waits(nc, limit=1)
    return nc


def _prep_core_inputs(c, x, Wq, Wdkv, Wuk, Wuv, Wkrope, Wqrope, bqrope,
                      ln_gamma, ln_beta, Wo, with_qbias):
    bf = ml_dtypes.bfloat16
    R = np.zeros((P, P), np.float32)
    R[np.arange(64), np.arange(64) + 64] = -1.0
    R[np.arange(64) + 64, np.arange(64)] = 1.0

    inv = (10000.0 ** (-np.arange(0, HD, 2, dtype=np.float32) / HD)).astype(np.float32)
    t = np.arange(S, dtype=np.float32)
    freqs = t[:, None] * inv[None, :]
    emb = np.concatenate([freqs, freqs], axis=-1)
    cosT = np.ascontiguousarray(np.cos(emb).T.astype(np.float32))
    sinT = np.ascontiguousarray(np.sin(emb).T.astype(np.float32))

    xt = np.empty((NU, P, S), np.float32)
    ws = np.empty((HPC, 7, P, P), np.float32)
    gb = np.empty((HPC, 2, P, 1), np.float32)
    qb = np.empty((HPC, 2, P, 1), np.float32)
    for hl in range(HPC):
        h = HPC * c + hl
        for b in range(B):
            xt[hl * B + b] = x[b, :, h * HD:(h + 1) * HD].T
        Wdkv_c = Wdkv[h] - Wdkv[h].mean(axis=0, keepdims=True)
        ws[hl, 0] = Wdkv_c.T
        ws[hl, 1] = Wkrope[h].T
        ws[hl, 2] = (R @ Wkrope[h]).T
        ws[hl, 3] = Wq[h] @ Wuk[h].T
        ws[hl, 4] = Wqrope[h].T
        ws[hl, 5] = (R @ Wqrope[h]).T
        ws[hl, 6] = Wuv[h] @ Wo[h].T
        gb[hl, 0, :, 0] = ln_gamma[h]
        gb[hl, 1, :, 0] = ln_beta[h]
        qb[hl, 0, :, 0] = bqrope[h]
        qb[hl, 1, :, 0] = R @ bqrope[h]

    im = {
        "xt": np.ascontiguousarray(xt),
        "ws": np.ascontiguousarray(ws),
        "gb": gb,
        "cosT": cosT,
        "sinT": sinT,
        "tri": (MASK_NEG * np.triu(np.ones((P, P), np.float32), 1)).astype(bf),
        "idb": np.eye(P, dtype=np.float32).astype(bf),
        "idr": np.eye(P, dtype=np.float32),
        "ones": np.ones((P, 1), np.float32),
    }
    if with_qbias:
        im["qb"] = qb
    return im


def kernel(x, Wq, Wdkv, Wuk, Wuv, Wkrope, Wqrope, bqrope, ln_gamma, ln_beta, Wo,
           _trace=False):
    with_qbias = bool(np.any(bqrope))
    key = with_qbias
    if key not in _prog_cache:
        _prog_cache[key] = _build(with_qbias)
    nc = _prog_cache[key]

    in_maps = [
        _prep_core_inputs(c, x, Wq, Wdkv, Wuk, Wuv, Wkrope, Wqrope, bqrope,
                          ln_gamma, ln_beta, Wo, with_qbias)
        for c in range(NCORES)
    ]
    res = run_bass_kernel_spmd(nc, in_maps, list(range(NCORES)), trace=_trace)
    kernel._last_result = res

    concat = np.empty((B, S, D), np.float32)
    ckv = np.empty((B, H, S, L), np.float32)
    kr = np.empty((B, H, S, L), np.float32)
    for c, r in enumerate(res.results):
        for hl in range(HPC):
            h = HPC * c + hl
            for b in range(B):
                u = hl * B + b
                concat[b, :, h * HD:(h + 1) * HD] = r["out_o"][u]
                ckv[b, h] = r["out_ckv"][u]
                kr[b, h] = r["out_kr"][u]
    return concat, ckv, kr


# revision 16
# speedup vs baseline: 68.6753x; 68.6753x over previous
"""MultiHeadLatentAttention TRN2 kernel.

Sharding: head-parallel across 8 NeuronCores (2 heads/core x 2 batches = 4
independent (batch, head) units per core). Everything on-device runs in a
"transposed" layout (feature dim on SBUF partitions, sequence on the free
dim) so scores are produced key-major and softmax numerators come straight
out of the Scalar engine in the layout the PV matmul needs.

Host-side algebraic folds (all exact):
  - LN mean:   Wdkv centered column-wise  -> ckv_raw is zero-mean over L.
  - rotate_half: premultiplied weight copies (R @ W) -> RoPE is two fused
    multiply-adds against precomputed cos/sin tables.
  - Wq @ Wuk^T absorbed (as in the reference), Wo folded into Wuv.
"""

import os
import sys

for _p in ("/opt/trn_rl_repo", "/root/.axon_site/_ro/trn_rl_repo"):
    if os.path.isdir(_p) and _p not in sys.path:
        sys.path.insert(0, _p)

import numpy as np
import ml_dtypes

import concourse.bass as bass
import concourse.mybir as mybir
from concourse.tile import TileContext
from concourse.bass_utils import run_bass_kernel_spmd

B, S, D, H, HD, L = 2, 2048, 2048, 16, 128, 128
P = 128
NCORES = 8
HPC = H // NCORES      # heads per core
NU = HPC * B           # (head, batch) units per core
NJ = S // P            # 128-row blocks along the sequence

F32 = mybir.dt.float32
F32R = mybir.dt.float32r
BF16 = mybir.dt.bfloat16
AF = mybir.ActivationFunctionType
ALU = mybir.AluOpType

MASK_NEG = -480.0      # causal mask additive constant, pre 1/16 scale
SCALE = 1.0 / 16.0     # 1/sqrt(2*L)

_prog_cache = {}


def _split_waits(nc, limit=1):
    """walrus enforces a small per-instruction sync-wait budget; move excess
    waits onto preceding NoOps on the same engine queue."""
    n = 0
    for f in nc.m.functions:
        for blk in f.blocks:
            out = []
            for inst in blk.instructions:
                si = inst.sync_info
                waits = list(si.on_wait) if si and si.on_wait else []
                if len(waits) > limit:
                    for k, w in enumerate(waits[:-limit]):
                        nop = mybir.InstNoOp(name=f"{inst.name}-ws{k}")
                        nop.engine = inst.engine
                        nop.sync_info = mybir.SyncInfo(on_wait=[w], on_update=[])
                        out.append(nop)
                        n += 1
                    si.on_wait = waits[-limit:]
                out.append(inst)
            blk.instructions = out
    return n


def _chunks(lo, hi, step):
    return [(o, min(step, hi - o)) for o in range(lo, hi, step)]


def _build(with_qbias=False):
    nc = bass.Bass(target_bir_lowering=False)

    xt_d = nc.dram_tensor("xt", [NU, P, S], F32R, kind="ExternalInput")
    ws_d = nc.dram_tensor("ws", [HPC, 7, P, P], F32R, kind="ExternalInput")
    gb_d = nc.dram_tensor("gb", [HPC, 2, P, 1], F32, kind="ExternalInput")
    cos_d = nc.dram_tensor("cosT", [P, S], F32, kind="ExternalInput")
    sin_d = nc.dram_tensor("sinT", [P, S], F32, kind="ExternalInput")
    tri_d = nc.dram_tensor("tri", [P, P], BF16, kind="ExternalInput")
    idb_d = nc.dram_tensor("idb", [P, P], BF16, kind="ExternalInput")
    one_d = nc.dram_tensor("ones", [P, 1], F32R, kind="ExternalInput")
    wuvb_d = nc.dram_tensor("wuvb", [HPC, P, P], BF16, kind="ExternalInput")
    if with_qbias:
        qb_d = nc.dram_tensor("qb", [HPC, 2, P, 1], F32, kind="ExternalInput")

    if timing:
        oo_d = nc.dram_tensor("out_o", [NU, P, S], F32)
        oc_d = nc.dram_tensor("out_ckv", [NU, P, S], F32R)
        ok_d = nc.dram_tensor("out_kr", [NU, P, S], F32R)
        tiny_d = nc.dram_tensor("tiny", [P, 1], F32, kind="ExternalOutput")
    else:
        oo_d = nc.dram_tensor("out_o", [NU, P, S], F32, kind="ExternalOutput")
        oc_d = nc.dram_tensor("out_ckv", [NU, P, S], F32R, kind="ExternalOutput")
        ok_d = nc.dram_tensor("out_kr", [NU, P, S], F32R, kind="ExternalOutput")
    op_d = nc.dram_tensor("out_pt", [NU, P, NJ], BF16, kind="ExternalOutput") \
        if phases == 2 else None

    with TileContext(nc) as tc:
        with tc.tile_pool(name="const", bufs=1) as cst, \
             tc.tile_pool(name="sb", bufs=2) as sb, \
             tc.tile_pool(name="sb1", bufs=1) as sb1, \
             tc.tile_pool(name="pt", bufs=1) as ptp, \
             tc.tile_pool(name="st", bufs=2) as st, \
             tc.tile_pool(name="psc", bufs=2, space="PSUM") as psc, \
             tc.tile_pool(name="pp", bufs=3, space="PSUM") as pp, \
             tc.tile_pool(name="po", bufs=1, space="PSUM") as po:

            # ---- constants ----
            cosT = cst.tile([P, S], F32)
            sinT = cst.tile([P, S], F32)
            tri = cst.tile([P, P], BF16)
            idb = cst.tile([P, P], BF16)
            ones = cst.tile([P, 1], F32R)
            onesrow = cst.tile([1, P], F32)
            nc.vector.memset(onesrow[:], 1.0)
            eps = cst.tile([1, 1], F32)
            nc.vector.memset(eps[:], 1e-5)
            nc.sync.dma_start(out=cosT[:], in_=cos_d[:])
            nc.sync.dma_start(out=sinT[:], in_=sin_d[:])
            nc.sync.dma_start(out=tri[:], in_=tri_d[:])
            nc.sync.dma_start(out=idb[:], in_=idb_d[:])
            nc.sync.dma_start(out=ones[:], in_=one_d[:])

            W = {}
            wuvb = {}
            gam, bet = {}, {}
            qb1, qb2 = {}, {}
            wnames = ["dkv", "kr", "krr", "abs", "qr", "qrr", "uv2"]
            for hl in range(HPC):
                for wi, wn in enumerate(wnames):
                    t = cst.tile([P, P], F32R, tag=f"w{hl}{wn}")
                    nc.sync.dma_start(out=t[:], in_=ws_d[hl, wi])
                    W[hl, wn] = t
                wuvb[hl] = cst.tile([P, P], BF16, tag=f"wb{hl}", name=f"wb{hl}")
                nc.sync.dma_start(out=wuvb[hl][:], in_=wuvb_d[hl])
                gam[hl] = cst.tile([P, 1], F32, tag=f"g{hl}", name=f"g{hl}")
                bet[hl] = cst.tile([P, 1], F32, tag=f"b{hl}", name=f"b{hl}")
                nc.sync.dma_start(out=gam[hl][:], in_=gb_d[hl, 0])
                nc.sync.dma_start(out=bet[hl][:], in_=gb_d[hl, 1])
                if with_qbias:
                    qb1[hl] = cst.tile([P, 1], F32, tag=f"q1{hl}", name=f"q1{hl}")
                    qb2[hl] = cst.tile([P, 1], F32, tag=f"q2{hl}", name=f"q2{hl}")
                    nc.sync.dma_start(out=qb1[hl][:], in_=qb_d[hl, 0])
                    nc.sync.dma_start(out=qb2[hl][:], in_=qb_d[hl, 1])

            for u in range(NU):
                hl, b = u // B, u % B

                # ---------- phase 1: load + projections ----------
                xts = sb.tile([P, S], F32R, tag="xts")
                nc.sync.dma_start(out=xts[:], in_=xt_d[u])

                ckvT = sb.tile([P, S], F32R, tag="ckvT")
                krT = sb.tile([P, S], F32R, tag="krT")
                q1T = sb.tile([P, S], F32R, tag="q1T")
                qrT = sb.tile([P, S], F32R, tag="qrT")

                # ckv chain with LayerNorm (mean pre-folded into weights)
                for off, w in _chunks(0, S, 512):
                    xc = pp.tile([P, 512], F32, tag="pp")[:, :w]
                    nc.tensor.matmul(xc, W[hl, "dkv"][:], xts[:, off:off + w],
                                     start=True, stop=True)
                    sq = st.tile([P, 512], F32R, tag="scr")[:, :w]
                    nc.scalar.activation(sq, xc, AF.Square)
                    var = pp.tile([P, 512], F32, tag="pp")[0:1, :w]
                    nc.tensor.matmul(var, ones[:], sq, start=True, stop=True)
                    lnv = st.tile([1, 512], F32, tag="ln")[:, :w]
                    nc.scalar.activation(lnv, var, AF.Ln, scale=1.0 / L, bias=eps[:])
                    rst = st.tile([1, 512], F32, tag="ln")[:, :w]
                    nc.scalar.activation(rst, lnv, AF.Exp, scale=-0.5)
                    rbp = pp.tile([P, 512], F32, tag="pp")[:, :w]
                    nc.tensor.matmul(rbp, onesrow[:], rst, start=True, stop=True)
                    rbc = st.tile([P, 512], F32, tag="rbc")[:, :w]
                    nc.vector.tensor_copy(rbc, rbp)
                    t1 = st.tile([P, 512], F32, tag="scr")[:, :w]
                    nc.vector.tensor_tensor(out=t1, in0=xc, in1=rbc, op=ALU.mult)
                    nc.vector.tensor_scalar(out=ckvT[:, off:off + w], in0=t1,
                                            scalar1=gam[hl][:], scalar2=bet[hl][:],
                                            op0=ALU.mult, op1=ALU.add)

                # decoupled RoPE key
                for off, w in _chunks(0, S, 512):
                    ka = pp.tile([P, 512], F32, tag="pp")[:, :w]
                    nc.tensor.matmul(ka, W[hl, "kr"][:], xts[:, off:off + w],
                                     start=True, stop=True)
                    kb = pp.tile([P, 512], F32, tag="pp")[:, :w]
                    nc.tensor.matmul(kb, W[hl, "krr"][:], xts[:, off:off + w],
                                     start=True, stop=True)
                    k1 = st.tile([P, 512], F32, tag="k1")[:, :w]
                    nc.vector.tensor_tensor(out=k1, in0=ka, in1=cosT[:, off:off + w],
                                            op=ALU.mult)
                    k2 = st.tile([P, 512], F32, tag="k2")[:, :w]
                    nc.vector.tensor_tensor(out=k2, in0=kb, in1=sinT[:, off:off + w],
                                            op=ALU.mult)
                    nc.vector.tensor_tensor(out=krT[:, off:off + w], in0=k1, in1=k2,
                                            op=ALU.add)

                # absorbed no-rope query
                for off, w in _chunks(0, S, 512):
                    qp = pp.tile([P, 512], F32, tag="pp")[:, :w]
                    nc.tensor.matmul(qp, W[hl, "abs"][:], xts[:, off:off + w],
                                     start=True, stop=True)
                    nc.vector.tensor_copy(q1T[:, off:off + w], qp)

                # RoPE query from latent
                for off, w in _chunks(0, S, 512):
                    qa = pp.tile([P, 512], F32, tag="pp")[:, :w]
                    nc.tensor.matmul(qa, W[hl, "qr"][:], ckvT[:, off:off + w],
                                     start=True, stop=True)
                    qb = pp.tile([P, 512], F32, tag="pp")[:, :w]
                    nc.tensor.matmul(qb, W[hl, "qrr"][:], ckvT[:, off:off + w],
                                     start=True, stop=True)
                    if with_qbias:
                        qa2 = st.tile([P, 512], F32, tag="k1")[:, :w]
                        nc.vector.tensor_scalar(out=qa2, in0=qa, scalar1=qb1[hl][:],
                                                scalar2=None, op0=ALU.add)
                        qb2s = st.tile([P, 512], F32, tag="k2")[:, :w]
                        nc.vector.tensor_scalar(out=qb2s, in0=qb, scalar1=qb2[hl][:],
                                                scalar2=None, op0=ALU.add)
                        qa, qb = qa2, qb2s
                    q1 = st.tile([P, 512], F32, tag="k1")[:, :w]
                    nc.vector.tensor_tensor(out=q1, in0=qa, in1=cosT[:, off:off + w],
                                            op=ALU.mult)
                    q2 = st.tile([P, 512], F32, tag="k2")[:, :w]
                    nc.vector.tensor_tensor(out=q2, in0=qb, in1=sinT[:, off:off + w],
                                            op=ALU.mult)
                    nc.vector.tensor_tensor(out=qrT[:, off:off + w], in0=q1, in1=q2,
                                            op=ALU.add)

                # values (Wo folded in) + ones column for the softmax denominator
                v2e = sb.tile([P, NJ * 129], BF16, tag="v2e")
                nc.vector.memset(v2e[:], 1.0)
                for j in range(NJ):
                    vp = pp.tile([P, 512], F32, tag="pp")[:, :P]
                    nc.tensor.matmul(vp, ckvT[:, j * P:(j + 1) * P],
                                     W[hl, "uv2"][:], start=True, stop=True)
                    nc.vector.tensor_copy(v2e[:, j * 129:j * 129 + P], vp)

                # natural-layout ckv / k_r outputs via PE transpose
                ckvn = sb1.tile([P, S], F32, tag="ckvn")
                krn = sb1.tile([P, S], F32, tag="krn")
                for src, dst in ((ckvT, ckvn), (krT, krn)):
                    for off, w in _chunks(0, S, 512):
                        tp = pp.tile([P, 512], F32R, tag="pp")[:, :w]
                        for k in range(0, w, P):
                            nc.tensor.transpose(tp[:, k:k + P],
                                                src[:, off + k:off + k + P], idr[:])
                        nc.vector.tensor_copy(dst[:, off:off + w], tp)

                # ---------- phase 2: attention ----------
                pts = []
                for j in range(NJ):
                    wsc = S - j * P
                    ptj = ptp.tile([P, wsc], BF16, tag=f"pt{j}")
                    pts.append(ptj)
                    for soff, sw in _chunks(j * P, S, 1024):
                        sc = psc.tile([P, 1024], F32, tag="sc")[:, :sw]
                        has_diag = soff == j * P
                        for o2, w2 in _chunks(soff, soff + sw, 512):
                            rel = o2 - soff
                            last = o2 + w2 == soff + sw
                            nc.tensor.matmul(sc[:, rel:rel + w2],
                                             ckvT[:, j * P:(j + 1) * P],
                                             q1T[:, o2:o2 + w2],
                                             start=True, stop=False,
                                             skip_group_check=True)
                            nc.tensor.matmul(sc[:, rel:rel + w2],
                                             krT[:, j * P:(j + 1) * P],
                                             qrT[:, o2:o2 + w2],
                                             start=False,
                                             stop=(not has_diag or rel > 0) and last,
                                             skip_group_check=True)
                        if has_diag:
                            nc.tensor.matmul(sc[:, 0:P], tri[:], idb[:],
                                             start=False, stop=True,
                                             skip_group_check=True)
                        nc.scalar.activation(ptj[:, soff - j * P:soff - j * P + sw],
                                             sc, AF.Exp, scale=SCALE)

                    # attention output + Z for query block i = j
                    op = pp.tile([P, 512], F32, tag="pp")[:, :129]
                    for jj in range(j + 1):
                        nc.tensor.matmul(op, pts[jj][:, (j - jj) * P:(j - jj + 1) * P],
                                         v2e[:, jj * 129:(jj + 1) * 129],
                                         start=(jj == 0), stop=(jj == j))
                    rz = st.tile([P, 1], F32, tag="rz")
                    nc.vector.reciprocal(rz[:], op[:, 128:129])
                    if j == 0:
                        ostg = sb1.tile([P, S], F32, tag="ostg")
                    nc.vector.tensor_scalar(out=ostg[:, j * P:(j + 1) * P],
                                            in0=op[:, 0:P], scalar1=rz[:],
                                            scalar2=None, op0=ALU.mult)

                # ---------- phase 3: stores ----------
                for stg, dd in ((ostg, oo_d), (ckvn, oc_d), (krn, ok_d)):
                    nc.sync.dma_start(
                        out=dd[u].rearrange("(j p) e -> p j e", p=P),
                        in_=stg[:].rearrange("p (j e) -> p j e", j=NJ))

    _split_waits(nc, limit=1)
    return nc


def _prep_core_inputs(c, x, Wq, Wdkv, Wuk, Wuv, Wkrope, Wqrope, bqrope,
                      ln_gamma, ln_beta, Wo, with_qbias):
    bf = ml_dtypes.bfloat16
    R = np.zeros((P, P), np.float32)
    R[np.arange(64), np.arange(64) + 64] = -1.0
    R[np.arange(64) + 64, np.arange(64)] = 1.0

    inv = (10000.0 ** (-np.arange(0, HD, 2, dtype=np.float32) / HD)).astype(np.float32)
    t = np.arange(S, dtype=np.float32)
    freqs = t[:, None] * inv[None, :]
    emb = np.concatenate([freqs, freqs], axis=-1)
    cosT = np.ascontiguousarray(np.cos(emb).T.astype(np.float32))
    sinT = np.ascontiguousarray(np.sin(emb).T.astype(np.float32))

    xt = np.empty((NU, P, S), np.float32)
    ws = np.empty((HPC, 7, P, P), np.float32)
    gb = np.empty((HPC, 2, P, 1), np.float32)
    qb = np.empty((HPC, 2, P, 1), np.float32)
    for hl in range(HPC):
        h = HPC * c + hl
        for b in range(B):
            xt[hl * B + b] = x[b, :, h * HD:(h + 1) * HD].T
        Wdkv_c = Wdkv[h] - Wdkv[h].mean(axis=0, keepdims=True)
        ws[hl, 0] = Wdkv_c.T
        ws[hl, 1] = Wkrope[h].T
        ws[hl, 2] = (R @ Wkrope[h]).T
        ws[hl, 3] = Wq[h] @ Wuk[h].T
        ws[hl, 4] = Wqrope[h].T
        ws[hl, 5] = (R @ Wqrope[h]).T
        ws[hl, 6] = Wuv[h] @ Wo[h].T
        gb[hl, 0, :, 0] = ln_gamma[h]
        gb[hl, 1, :, 0] = ln_beta[h]
        qb[hl, 0, :, 0] = bqrope[h]
        qb[hl, 1, :, 0] = R @ bqrope[h]

    im = {
        "xt": np.ascontiguousarray(xt),
        "ws": np.ascontiguousarray(ws),
        "gb": gb,
        "cosT": cosT,
        "sinT": sinT,
        "tri": (MASK_NEG * np.triu(np.ones((P, P), np.float32), 1)).astype(bf),
        "idb": np.eye(P, dtype=np.float32).astype(bf),
        "ones": np.ones((P, 1), np.float32),
        "wuvb": ws[:, 6].astype(bf),
    }
    if with_qbias:
        im["qb"] = qb
    return im


def kernel(x, Wq, Wdkv, Wuk, Wuv, Wkrope, Wqrope, bqrope, ln_gamma, ln_beta, Wo,
           _trace=False):
    with_qbias = bool(np.any(bqrope))
    key = with_qbias
    if key not in _prog_cache:
        _prog_cache[key] = _build(with_qbias)
    nc = _prog_cache[key]

    in_maps = [
        _prep_core_inputs(c, x, Wq, Wdkv, Wuk, Wuv, Wkrope, Wqrope, bqrope,
                          ln_gamma, ln_beta, Wo, with_qbias)
        for c in range(NCORES)
    ]
    import time as _time
    _t0 = _time.time()
    res = run_bass_kernel_spmd(nc, in_maps, list(range(NCORES)), trace=_trace)
    kernel._last_run_s = _time.time() - _t0
    kernel._last_result = res

    concat = np.empty((B, S, D), np.float32)
    ckv = np.empty((B, H, S, L), np.float32)
    kr = np.empty((B, H, S, L), np.float32)
    for c, r in enumerate(res.results):
        for hl in range(HPC):
            h = HPC * c + hl
            for b in range(B):
                u = hl * B + b
                # out_o: (p, j*P+e) -> (j*P+p, e)
                oo = r["out_o"][u].reshape(P, NJ, P).transpose(1, 0, 2)
                concat[b, :, h * HD:(h + 1) * HD] = oo.reshape(S, P)
                ckv[b, h] = r["out_ckv"][u].T
                kr[b, h] = r["out_kr"][u].T
    return concat, ckv, kr
